# revision 54
# baseline (speedup 1.0000x reference)
"""Gated Slot Attention (GSA) Trainium2 kernel, v7.

Sharding: B*H = 8 lanes -> 8 cores (core = b*4 + h). Each core computes its
lane's projections + chunked two-pass GLA recurrence, emitting the raw lane
output z = 256*o transposed [DV, T]. A second kernel applies silu + RMSNorm +
output projection with rows of (b,t) split across cores.

Key design points (cost-model driven, hardware-verified):
  - q/k/f projections run in fp8e4m3 with perf_mode=DoubleRow (weights
    host-prescaled by 128 to clear the e4m3 subnormal range; the scales are
    folded into the tanh/exp activation scales and the softmax exp scale).
  - v projection is fp8 DoubleRow with residual correction: three terms
    hs8@wv8 + hr8@wv4 + hs8@wvr8 accumulate in one psum at scale 128;
    the T1+T3 pair cancels the wv quantization exactly, hr8 carries the
    hs quantization residual (x32). Measured ~1.1e-3 on v vs 2.6e-2 for
    naive fp8 - the value path is the error-critical one.
  - chunk pipeline stages P(transposes+state), Q(logits+exp), R(softmax
    denom), S(pass-2 output) run at lags 0/1/2/3, interleaved with the
    later projection batches so the PE queue never blocks on input DMAs.
  - the softmax denominator factors out of both pass-2 matmul groups
    (they are linear in qtt), so S consumes the unnormalized lam*et and
    the reciprocal-denominator broadcast multiplies the z output tile.
  - Lam_end broadcasts built batched via diag trick: diag = ident*lend_col
    (per-partition tensor_scalar), then ones^T @ diag broadcasts along free.
  - mask applied blockwise: only the two diagonal 128x128 blocks need the
    triangular multiply (in-place on SBUF after one Act copy); the zero
    block is skipped via partial-psum accumulation in the consumers.
  - chunk psums are merged into [128,512] banks ([Hk-half | Hv-half],
    [ok mt0 | ok mt1], ...) so each psum->sbuf crossing is one op.
  - GPSIMD (Pool) cannot touch PSUM and cannot run TensorScalarPtr-class
    ops (scan/stt/tensor_scalar) on this compiler: Pool only gets SBUF
    TensorTensor/copies (stb subtract, tmp2 = lam*et, p2m tri masks).

Chunked recurrence (C=256, all within one lane):
  Lam[i,m] = prod_{j<=i} g[j,m]  (= exp(-cumsum(softplus(-xf))/8))
  rlam = 1/Lam ; st_t = s_t/Lam_t = rlam_t - rlam_{t-1}
  ok   = Lam*(q @ Hk + mask(k^T q)^T St); qv = softmax_m(ok); qtt = qv*Lam
  o    = qtt @ Hv + mask(St qtt)^T v
  Hk' = Lend*(Hk + k^T St) ; Hv' = Lend*(Hv + St^T v)   (Lend pulled out)

silu is synthesized as 2*silu(x) = (tanh(x/2)+1)*x on the 128x-prescaled
psums, so q,k,v all carry 256*silu; q*k's 2^16 cancels in the exp scale,
v's 256 rides into kernel 2 where u = (tanh(z/512)+1)*z = 512*silu(o),
the RMSNorm scale uses 2^-28, and wot folds g_w/512.
"""
import sys
sys.path.insert(0, '/opt/trn_rl_repo')

import numpy as np
import ml_dtypes

import concourse.bass as bass
import concourse.bacc as bacc
import concourse.tile as tile
import concourse.mybir as mybir
import concourse.bass_utils as bass_utils

BF = mybir.dt.bfloat16
F32 = mybir.dt.float32
E4 = mybir.dt.float8e4
AF = mybir.ActivationFunctionType
OP = mybir.AluOpType
DR = mybir.MatmulPerfMode.DoubleRow

B, T, D = 2, 2048, 1024
H, DK, DV, M = 4, 256, 256, 256
C = 256            # chunk length
NCHUNK = T // C
NBATCH = NCHUNK // 2   # 2-chunk projection batches
GATE_NORM = 8.0
EPS = 1e-5
WS = 128.0         # fp8 weight prescale

_cache = {}


def build_gsa():
    """Kernel 1: per-lane projections + chunked GLA. Output z [256, 2048] bf16
    (= 2*o, feature-major)."""
    nc = bacc.Bacc("TRN2", target_bir_lowering=False, debug=False, num_devices=8)
    hs8_d = nc.dram_tensor("hs8", [D, T], E4, kind="ExternalInput").ap()
    hr8_d = nc.dram_tensor("hr8", [D, T], E4, kind="ExternalInput").ap()
    w8_d = nc.dram_tensor("w8", [D, 1536], E4, kind="ExternalInput").ap()
    mask_d = nc.dram_tensor("mask", [128, 128], BF, kind="ExternalInput").ap()
    ident_d = nc.dram_tensor("ident", [128, 128], BF, kind="ExternalInput").ap()
    z_d = nc.dram_tensor("z", [DV, T], BF, kind="ExternalOutput").ap()

    with tile.TileContext(nc) as tc:
        with (
            tc.tile_pool(name="persist", bufs=1) as pp,
            tc.tile_pool(name="hs8p", bufs=NBATCH) as hs8p,
            tc.tile_pool(name="hsbp", bufs=NBATCH) as hsbp,
            tc.tile_pool(name="gb", bufs=2) as gb,      # gate short-lived
            tc.tile_pool(name="gk", bufs=NBATCH) as gk,  # gate kept
            tc.tile_pool(name="qk", bufs=NBATCH) as qkp,
            tc.tile_pool(name="lv", bufs=8) as lv,      # per-chunk leaf tensors
            tc.tile_pool(name="l4", bufs=5) as l4,      # shorter-lived leafs
            tc.tile_pool(name="sn", bufs=6) as snp,     # state snapshots
            tc.tile_pool(name="wk", bufs=4) as wk,      # short-lived
            tc.tile_pool(name="p512", bufs=2, space="PSUM") as p512,
            tc.tile_pool(name="pC", bufs=3, space="PSUM") as pC,
            tc.tile_pool(name="pT", bufs=2, space="PSUM") as pT,
            tc.tile_pool(name="pR", bufs=1, space="PSUM") as pR,
        ):
            w8 = pp.tile([128, 8, 1536], E4, tag="w8")
            msk = pp.tile([128, 128], BF, tag="msk")
            ident = pp.tile([128, 128], BF, tag="ident")
            ones_col = pp.tile([128, 1], BF, tag="onescol")
            ones_row = pp.tile([1, 128], BF, tag="onesrow")
            ones_mat = pp.tile([128, 128], BF, tag="onesmat")
            ones_2h = pp.tile([128, 256], BF, tag="ones2h")
            e1all = pp.tile([128, 8, 512], F32, tag="e1all")

            w8v = w8_d.rearrange("(a p) o -> p a o", p=128)
            hsv8 = hs8_d.rearrange("(a p) t -> p a t", p=128)
            hsv = hr8_d.rearrange("(a p) t -> p a t", p=128)
            # f weights first: the gate phase runs before everything else.
            nc.sync.dma_start(out=w8[:, :, 512:768], in_=w8v[:, :, 512:768])
            hs8_t, hsb_t = {}, {}
            for bt in range(NBATCH):
                hs8_t[bt] = hs8p.tile([128, 8, 512], E4, tag="hs8", name="hs8")
            for bt in range(NBATCH):
                hsb_t[bt] = hsbp.tile([128, 8, 512], E4, tag="hr8", name="hr8")
            nc.sync.dma_start(out=hs8_t[0], in_=hsv8[:, :, 0:512])
            nc.sync.dma_start(out=w8[:, :, 0:512], in_=w8v[:, :, 0:512])
            for bt in range(1, NBATCH):
                nc.sync.dma_start(out=hs8_t[bt],
                                  in_=hsv8[:, :, bt * 512:(bt + 1) * 512])
            nc.sync.dma_start(out=msk, in_=mask_d)
            nc.sync.dma_start(out=ident, in_=ident_d)
            nc.sync.dma_start(out=w8[:, :, 768:1536], in_=w8v[:, :, 768:1536])
            for bt in range(NBATCH):
                nc.sync.dma_start(out=hsb_t[bt],
                                  in_=hsv[:, :, bt * 512:(bt + 1) * 512])
            nc.vector.memset(ones_col, 1.0)
            nc.vector.memset(ones_row, 1.0)
            nc.gpsimd.memset(ones_mat, 1.0)
            nc.gpsimd.memset(ones_2h, 1.0)
            # warm the PE pstate during the initial DMA wait: tiny matmuls
            # keep the tensor engine busy so the first real projections run
            # at full clock instead of the mid pstate
            wps = pR.tile([128, 512], F32, tag="pR", name="wps")
            for _ in range(40):
                nc.tensor.matmul(wps[0:1, 0:1], lhsT=ones_col, rhs=ones_col,
                                 start=True, stop=True, skip_group_check=True)

            zv = z_d.rearrange("(a p) t -> p a t", p=128)

            lendf = pp.tile([128, 2, NCHUNK], F32, tag="lendf")
            lamb, stb, qtb, ktb = {}, {}, {}, {}
            v_un, st_un, k_un, hxb, et, tmp2, qtt = ({} for _ in range(7))

            # ---- phase F: f projections (fp8 DoubleRow) + exp for all
            # batches; then one Ln for softplus; then per-batch gate math.
            for bt in range(NBATCH):
                hs8 = hs8_t[bt]
                for mt in range(2):
                    ps = p512.tile([128, 512], F32, tag="p512")
                    for d2 in range(4):
                        nc.tensor.matmul(
                            ps,
                            lhsT=w8[:, 2 * d2:2 * d2 + 2,
                                    512 + mt * 128:512 + (mt + 1) * 128],
                            rhs=hs8[:, 2 * d2:2 * d2 + 2, :],
                            perf_mode=DR, start=(d2 == 0), stop=(d2 == 3))
                    # e1 = exp(-xf); psum carries 128*xf
                    nc.scalar.activation(e1all[:, bt * 2 + mt, :], ps, AF.Exp,
                                         scale=-1.0 / WS)
            # ln(e1 + 1) = softplus(-xf) = nsp, all batches in one instruction
            nc.scalar.activation(e1all, e1all, AF.Ln, bias=1.0)

            def stage_F3(bt):
                """Post-ln gate math for one batch: cumsum, rlam, lam, st."""
                e1 = e1all[:, bt * 2:bt * 2 + 2, :]
                rl = gb.tile([128, 2, 512], F32, tag="rl", name="rl")
                Sb = gb.tile([128, 2, 512], F32, tag="Sb", name="Sb")
                lamb[bt] = gk.tile([128, 2, 512], BF, tag="lamb", name="lamb")
                stb[bt] = gk.tile([128, 2, 512], BF, tag="stb", name="stb")
                seng = nc.vector
                for mt in range(2):
                    # per-chunk cumsum of nsp
                    seng.tensor_tensor_scan(
                        Sb[:, mt, 0:256], e1[:, mt, 0:256], e1[:, mt, 0:256],
                        0.0, OP.add, OP.bypass)
                    seng.tensor_tensor_scan(
                        Sb[:, mt, 256:512], e1[:, mt, 256:512],
                        e1[:, mt, 256:512], 0.0, OP.add, OP.bypass)
                # rlam = exp(cumsum/8), both mt in one activation
                nc.scalar.activation(
                    rl.rearrange("p a b -> p (a b)"),
                    Sb.rearrange("p a b -> p (a b)"), AF.Exp,
                    scale=1.0 / GATE_NORM)
                with nc.allow_low_precision(reason="lam in bf16"):
                    nc.vector.reciprocal(
                        lamb[bt].rearrange("p a b -> p (a b)"),
                        rl.rearrange("p a b -> p (a b)"))
                for mt in range(2):
                    for h2 in range(2):
                        nc.vector.tensor_copy(
                            lendf[:, mt, 2 * bt + h2:2 * bt + h2 + 1],
                            lamb[bt][:, mt, h2 * 256 + 255:h2 * 256 + 256])
                    # st_t = rlam_t - rlam_{t-1}; chunk-boundary cols use rlam=1
                    nc.gpsimd.tensor_tensor(
                        stb[bt][:, mt, 1:512], rl[:, mt, 1:512],
                        rl[:, mt, 0:511], op=OP.subtract)
                    for h2 in range(2):
                        nc.vector.tensor_scalar_sub(
                            stb[bt][:, mt, h2 * 256:h2 * 256 + 1],
                            rl[:, mt, h2 * 256:h2 * 256 + 1], 1.0)


            def chunk_views(c):
                bt, h2 = c // 2, c % 2
                off = h2 * 256
                stc = stb[bt][:, :, off:off + 256]
                lamc = lamb[bt][:, :, off:off + 256]
                qtc = qtb[bt][:, :, off:off + 256]
                ktc = ktb[bt][:, :, off:off + 256]
                return stc, lamc, qtc, ktc

            # ---- phase G: q/k (fp8 DoubleRow) + v (bf16) projections + silu.
            def stage_Gqk(bt):
                hs8 = hs8_t[bt]
                qtb[bt] = qkp.tile([128, 2, 512], BF, tag="qtb", name="qtb")
                ktb[bt] = qkp.tile([128, 2, 512], BF, tag="ktb", name="ktb")
                for base, dst in ((0, qtb[bt]), (256, ktb[bt])):
                    for ot in range(2):
                        ps = p512.tile([128, 512], F32, tag="p512")
                        for d2 in range(4):
                            nc.tensor.matmul(
                                ps,
                                lhsT=w8[:, 2 * d2:2 * d2 + 2,
                                        base + ot * 128:base + (ot + 1) * 128],
                                rhs=hs8[:, 2 * d2:2 * d2 + 2, :],
                                perf_mode=DR, start=(d2 == 0), stop=(d2 == 3))
                        th = wk.tile([128, 512], BF, tag="th")
                        # psum = 128*x -> th = tanh(x/2); dst = (th+1)*psum
                        # = 256*silu(x)
                        nc.scalar.activation(th, ps, AF.Tanh, scale=0.5 / WS)
                        nc.vector.scalar_tensor_tensor(
                            out=dst[:, ot, :], in0=th, scalar=1.0, in1=ps,
                            op0=OP.add, op1=OP.mult)
            def stage_Gv(bt):
                """v = hs8@wv8 + hr8@wv8b + hs8@wvr8, all fp8 DoubleRow at
                psum scale 128 (T1+T3 cancel the wv quantization exactly)."""
                hs8, hr8 = hs8_t[bt], hsb_t[bt]
                for h2 in range(2):
                    c = 2 * bt + h2
                    v_un[c] = lv.tile([128, 2, 256], BF, tag="vun", name="vun",
                                      bufs=8)
                    psw = p512.tile([128, 512], F32, tag="p512")
                    for tt in range(2):
                        ps = psw[:, tt * 256:(tt + 1) * 256]
                        tsl = slice(h2 * 256 + tt * 128, h2 * 256 + (tt + 1) * 128)
                        for d2 in range(4):
                            d2s = slice(2 * d2, 2 * d2 + 2)
                            nc.tensor.matmul(
                                ps, lhsT=hs8[:, d2s, tsl],
                                rhs=w8[:, d2s, 768:1024], perf_mode=DR,
                                start=(d2 == 0), stop=False,
                                skip_group_check=True)
                        for d2 in range(4):
                            d2s = slice(2 * d2, 2 * d2 + 2)
                            nc.tensor.matmul(
                                ps, lhsT=hr8[:, d2s, tsl],
                                rhs=w8[:, d2s, 1024:1280], perf_mode=DR,
                                start=False, stop=False, skip_group_check=True)
                        for d2 in range(4):
                            d2s = slice(2 * d2, 2 * d2 + 2)
                            nc.tensor.matmul(
                                ps, lhsT=hs8[:, d2s, tsl],
                                rhs=w8[:, d2s, 1280:1536], perf_mode=DR,
                                start=False, stop=(d2 == 3),
                                skip_group_check=True)
                    th = wk.tile([128, 512], BF, tag="th")
                    # psum = 128*v -> v_un = 256*silu(v)
                    nc.scalar.activation(th, psw, AF.Tanh, scale=0.5 / WS)
                    nc.vector.scalar_tensor_tensor(
                        out=v_un[c].rearrange("p a b -> p (a b)"), in0=th,
                        scalar=1.0, in1=psw, op0=OP.add, op1=OP.mult)

            # Batched Lend broadcast: diag trick
            # dg[p,j] = ident[p,j]*Lend[p]; ones^T @ dg -> [128, Lend[j]].
            # One round covers the 2 chunks of one batch.
            lbc_all = pp.tile([128, NCHUNK - 1, 256], BF, tag="lbcall")

            def stage_prep(r):
                pbc = pR.tile([128, 512], F32, tag="pR")
                dgr = wk.tile([128, 4, 128], BF, tag="dgr")
                nch = min(2, NCHUNK - 1 - 2 * r)
                for i in range(nch):
                    c = 2 * r + i
                    for mt in range(2):
                        nc.vector.tensor_scalar_mul(
                            dgr[:, 2 * i + mt, :], ident,
                            lendf[:, mt, c:c + 1])
                        nc.tensor.matmul(
                            pbc[:, i * 256 + mt * 128:i * 256 + (mt + 1) * 128],
                            lhsT=ones_mat, rhs=dgr[:, 2 * i + mt, :],
                            start=True, stop=True, skip_group_check=True)
                nc.scalar.activation(
                    lbc_all[:, 2 * r:2 * r + nch, :].rearrange(
                        "p a b -> p (a b)"),
                    pbc[:, 0:nch * 256], AF.Copy)

            def stage_P(c):
                """Transposes, merged state updates."""
                stc, lamc, qtc, ktc = chunk_views(c)
                # transposes: [tau, m | dk]: skun[:,lt,0:256]=st_un, 256:512=k_un
                skun = lv.tile([128, 2, 512], BF, tag="skun", name="skun")
                st_un[c] = skun[:, :, 0:256]
                k_un[c] = skun[:, :, 256:512]
                pst = pT.tile([128, 1024], BF, tag="pT")
                for lt in range(2):
                    for mt in range(2):
                        nc.tensor.transpose(
                            pst[:, lt * 512 + mt * 128:lt * 512 + (mt + 1) * 128],
                            stc[:, mt, lt * 128:(lt + 1) * 128], ident)
                    for k2 in range(2):
                        nc.tensor.transpose(
                            pst[:, lt * 512 + 256 + k2 * 128:
                                lt * 512 + 256 + (k2 + 1) * 128],
                            ktc[:, k2, lt * 128:(lt + 1) * 128], ident)
                nc.scalar.activation(
                    skun.rearrange("p a b -> p (a b)"), pst, AF.Copy)

                if c >= NCHUNK - 1:
                    return
                # merged state psums: psx = [Hk(dt2=x) | Hv(mt=x)] [128,512]
                first = c == 0
                hxb[c] = [snp.tile([128, 512], BF, tag=f"hxb{x}", name="hxb")
                          for x in range(2)]
                for x in range(2):
                    ps = pC.tile([128, 512], F32, tag="pC")
                    for lt in range(2):
                        nc.tensor.matmul(
                            ps[:, 0:256],
                            lhsT=k_un[c][:, lt, x * 128:(x + 1) * 128],
                            rhs=st_un[c][:, lt, :], start=(lt == 0),
                            stop=False, skip_group_check=True)
                    for lt in range(2):
                        nc.tensor.matmul(
                            ps[:, 256:512],
                            lhsT=st_un[c][:, lt, x * 128:(x + 1) * 128],
                            rhs=v_un[c][:, lt, :], start=(lt == 0),
                            stop=(lt == 1 and first), skip_group_check=True)
                    if not first:
                        nc.tensor.matmul(ps, lhsT=ident, rhs=hxb[c - 1][x],
                                         start=False, stop=True,
                                         skip_group_check=True)
                    nc.vector.tensor_tensor(hxb[c][x][:, 0:256], ps[:, 0:256],
                                            lbc_all[:, c, :], op=OP.mult)
                    nc.vector.tensor_scalar_mul(hxb[c][x][:, 256:512],
                                                ps[:, 256:512],
                                                lendf[:, x, c:c + 1])

            def hk_v(c, k2):
                return hxb[c][k2][:, 0:256]

            def hv_v(c, mt):
                return hxb[c][mt][:, 256:512]

            def stage_Q(c):
                """Gram + masked intra + state ok + exp for chunk c (lag 1)."""
                stc, lamc, qtc, ktc = chunk_views(c)
                # ptm[lambda, tau] = mask * (k^T q); blockwise, merged psum.
                ptm = l4.tile([128, 2, 256], BF, tag="ptm", name="ptm")
                psg = pC.tile([128, 512], F32, tag="pC")
                for lt in range(2):
                    for k2 in range(2):
                        nc.tensor.matmul(
                            psg[:, lt * 256:(lt + 1) * 256],
                            lhsT=ktc[:, k2, lt * 128:(lt + 1) * 128],
                            rhs=qtc[:, k2, :], start=(k2 == 0), stop=(k2 == 1),
                            skip_group_check=True)
                nc.scalar.activation(
                    ptm.rearrange("p a b -> p (a b)"), psg, AF.Copy)
                nc.vector.tensor_tensor(ptm[:, 0, 0:128], ptm[:, 0, 0:128],
                                        msk, op=OP.mult)
                nc.vector.tensor_tensor(ptm[:, 1, 128:256], ptm[:, 1, 128:256],
                                        msk, op=OP.mult)
                tmp = wk.tile([128, 2, 256], F32, tag="tmp")
                et[c] = l4.tile([128, 2, 256], BF, tag="et", name="et")
                first = c == 0
                pso = pC.tile([128, 512], F32, tag="pC")
                for mt in range(2):
                    ob = pso[:, mt * 256:(mt + 1) * 256]
                    if not first:
                        for k2 in range(2):
                            nc.tensor.matmul(
                                ob, lhsT=hk_v(c - 1, k2)[:, mt * 128:(mt + 1) * 128],
                                rhs=qtc[:, k2, :], start=(k2 == 0), stop=False,
                                skip_group_check=True)
                    nc.tensor.matmul(
                        ob, lhsT=st_un[c][:, 0, mt * 128:(mt + 1) * 128],
                        rhs=ptm[:, 0, :], start=first, stop=False,
                        skip_group_check=True)
                    nc.tensor.matmul(
                        ob[:, 128:256],
                        lhsT=st_un[c][:, 1, mt * 128:(mt + 1) * 128],
                        rhs=ptm[:, 1, 128:256], start=False, stop=(mt == 1),
                        skip_group_check=True)
                nc.vector.tensor_tensor(
                    tmp, pso.rearrange("p (a b) -> p a b", a=2),
                    lamc, op=OP.mult)
                # q,k each carry 256x -> exp scale 2^-16
                nc.scalar.activation(
                    et[c].rearrange("p a b -> p (a b)"),
                    tmp.rearrange("p a b -> p (a b)"), AF.Exp,
                    scale=1.0 / 65536.0)
                # unnormalized qtt = lam * et (normalized in stage_R)
                tmp2[c] = l4.tile([128, 2, 256], BF, tag="tmp2", name="tmp2")
                nc.gpsimd.tensor_tensor(tmp2[c], lamc, et[c], op=OP.mult)

            def stage_R(c):
                """Softmax normalization for chunk c (lag 2)."""
                prt = pR.tile([128, 512], F32, tag="pR")
                cs = prt[0:1, 256:512]
                for mt in range(2):
                    nc.tensor.matmul(cs, lhsT=ones_col, rhs=et[c][:, mt, :],
                                     start=(mt == 0), stop=(mt == 1),
                                     skip_group_check=True)
                rrow = wk.tile([1, C], BF, tag="rrow")
                with nc.allow_low_precision(reason="softmax denom bcast in bf16"):
                    nc.vector.reciprocal(rrow, cs)
                bcr = prt[:, 0:256]
                nc.tensor.matmul(bcr, lhsT=ones_row, rhs=rrow, start=True,
                                 stop=True, skip_group_check=True)
                rbb = l4.tile([128, C], BF, tag="rbb", name="rbb")
                nc.vector.tensor_copy(rbb, bcr)
                qtt[c] = rbb  # denominator factor, applied to z in stage_S

            def stage_S(c):
                """Pass-2 output for chunk c (lag 3)."""
                stc, lamc, qtc, ktc = chunk_views(c)
                # p2m[lambda, tau] = mask * (St qtt); blockwise, merged psum.
                p2m = wk.tile([128, 2, C], BF, tag="p2m")
                ps2 = pC.tile([128, 512], F32, tag="pC")
                for mt in range(2):
                    nc.tensor.matmul(
                        ps2[:, 0:256], lhsT=stc[:, mt, 0:128],
                        rhs=tmp2[c][:, mt, :], start=(mt == 0), stop=False,
                        skip_group_check=True)
                for mt in range(2):
                    nc.tensor.matmul(
                        ps2[:, 384:512], lhsT=stc[:, mt, 128:256],
                        rhs=tmp2[c][:, mt, 128:256], start=(mt == 0),
                        stop=(mt == 1), skip_group_check=True)
                nc.scalar.activation(
                    p2m.rearrange("p a b -> p (a b)"), ps2, AF.Copy)
                nc.gpsimd.tensor_tensor(p2m[:, 0, 0:128], p2m[:, 0, 0:128],
                                        msk, op=OP.mult)
                nc.gpsimd.tensor_tensor(p2m[:, 1, 128:256], p2m[:, 1, 128:256],
                                        msk, op=OP.mult)

                first = c == 0
                zps = p512.tile([128, 512], F32, tag="p512")
                for vt in range(2):
                    zp = zps[:, vt * 256:(vt + 1) * 256]
                    nc.tensor.matmul(
                        zp, lhsT=v_un[c][:, 0, 128 * vt:128 * (vt + 1)],
                        rhs=p2m[:, 0, :], start=True, stop=False,
                        skip_group_check=True)
                    nc.tensor.matmul(
                        zp[:, 128:256],
                        lhsT=v_un[c][:, 1, 128 * vt:128 * (vt + 1)],
                        rhs=p2m[:, 1, 128:256], start=False,
                        stop=(first and vt == 1), skip_group_check=True)
                    if not first:
                        for mt in range(2):
                            nc.tensor.matmul(
                                zp, lhsT=hv_v(c - 1, mt)[:, vt * 128:(vt + 1) * 128],
                                rhs=tmp2[c][:, mt, :], start=False,
                                stop=(mt == 1), skip_group_check=True)
                # un-normalized z; the softmax denominator (rbb) factors out
                # of both matmul groups and is applied here
                zsb = wk.tile([128, 2, C], BF, tag="zsb")
                for vt in range(2):
                    nc.vector.tensor_tensor(
                        zsb[:, vt, :], zps[:, vt * 256:(vt + 1) * 256],
                        qtt[c], op=OP.mult)
                nc.sync.dma_start(out=zv[:, :, c * C:(c + 1) * C], in_=zsb)

            stage_F3(0)
            stage_prep(0)
            stage_Gqk(0)
            stage_Gv(0)
            stage_F3(1)
            stage_prep(1)
            stage_Gqk(1)
            for it in range(NCHUNK + 3):
                if it == 0:
                    stage_Gv(1)
                if it == 2:
                    stage_F3(2)
                    stage_prep(2)
                    stage_Gqk(2)
                    stage_Gv(2)
                if it == 4:
                    stage_F3(3)
                    stage_prep(3)
                    stage_Gqk(3)
                    stage_Gv(3)
                if it < NCHUNK:
                    stage_P(it)
                if 1 <= it < NCHUNK + 1:
                    stage_Q(it - 1)
                if 2 <= it < NCHUNK + 2:
                    stage_R(it - 2)
                if 3 <= it < NCHUNK + 3:
                    stage_S(it - 3)
    nc.compile()
    return nc


def build_final():
    """Kernel 2: z = 256*o; u = (tanh(z/512)+1)*z = 512*silu(o);
    y = (u * rsqrt-bcast) @ wot, wot host-folded (g_w, 1/512).
    Out yT [1024, 512] bf16."""
    nc = bacc.Bacc("TRN2", target_bir_lowering=False, debug=False, num_devices=8)
    z_d = nc.dram_tensor("zin", [D, 512], BF, kind="ExternalInput").ap()
    wo_d = nc.dram_tensor("wot", [D, D], BF, kind="ExternalInput").ap()
    y_d = nc.dram_tensor("y", [D, 512], BF, kind="ExternalOutput").ap()

    with tile.TileContext(nc) as tc:
        with (
            tc.tile_pool(name="sb", bufs=1) as sb,
            tc.tile_pool(name="yp", bufs=3) as yp,
            tc.tile_pool(name="ps", bufs=6, space="PSUM") as psp,
            tc.tile_pool(name="pss", bufs=1, space="PSUM") as pssp,
            tc.tile_pool(name="psb", bufs=1, space="PSUM") as psbp,
        ):
            z = sb.tile([128, 8, 512], BF, tag="z")
            wo = sb.tile([128, 8, 1024], BF, tag="wo")
            u = sb.tile([128, 8, 512], BF, tag="u")
            squ = sb.tile([128, 8, 512], BF, tag="squ")
            ysb0 = sb.tile([128, 4, 512], BF, tag="ysb0")
            ysb1 = sb.tile([128, 4, 512], BF, tag="ysb1")
            ones_col = sb.tile([128, 1], BF, tag="onescol")
            ones_row = sb.tile([1, 128], BF, tag="onesrow")
            nc.vector.memset(ones_col, 1.0)
            nc.vector.memset(ones_row, 1.0)
            zvw = z_d.rearrange("(a p) t -> p a t", p=128)
            wov = wo_d.rearrange("(a p) o -> p a o", p=128)
            for ct in range(8):
                nc.sync.dma_start(out=z[:, ct, :], in_=zvw[:, ct, :])
                nc.sync.dma_start(out=wo[:, ct, :], in_=wov[:, ct, :])

            # silu: z = 2*o ; u = (tanh(o/2)+1)*z = 4*silu(o)
            sqs = pssp.tile([1, 512], F32, tag="pss")
            yps = [psp.tile([128, 512], F32, tag="ps", name="yps")
                   for _ in range(4)]
            for ct in range(8):
                th = yp.tile([128, 512], BF, tag="th")
                nc.scalar.activation(th, z[:, ct, :], AF.Tanh,
                                     scale=1.0 / 512.0)
                nc.vector.scalar_tensor_tensor(
                    out=u[:, ct, :], in0=th, scalar=1.0, in1=z[:, ct, :],
                    op0=OP.add, op1=OP.mult)
                nc.vector.tensor_tensor(squ[:, ct, :], u[:, ct, :], u[:, ct, :],
                                        op=OP.mult)
                nc.tensor.matmul(sqs, lhsT=ones_col, rhs=squ[:, ct, :],
                                 start=(ct == 0), stop=(ct == 7))
                for oi in range(4):
                    nc.tensor.matmul(
                        yps[oi], lhsT=wo[:, ct, oi * 128:(oi + 1) * 128],
                        rhs=u[:, ct, :], start=(ct == 0), stop=(ct == 7))
            # u = 512*silu -> mean(silu^2) = sqs/(512^2*1024) = sqs*2^-28
            sq = sb.tile([1, 512], F32, tag="sq")
            eps_t = sb.tile([1, 1], F32, tag="epst")
            nc.vector.memset(eps_t, EPS)
            nc.scalar.activation(sq, sqs, AF.Sqrt, scale=2.0 ** -28,
                                 bias=eps_t)
            rr = sb.tile([1, 512], BF, tag="rr")
            with nc.allow_low_precision(reason="rms bcast in bf16"):
                nc.vector.reciprocal(rr, sq)
            bcr = psbp.tile([128, 512], F32, tag="psb")
            nc.tensor.matmul(bcr, lhsT=ones_row, rhs=rr, start=True, stop=True)
            rbb = sb.tile([128, 512], F32, tag="rbb")
            nc.vector.tensor_copy(rbb, bcr)

            yv = y_d.rearrange("(a p) t -> p a t", p=128)
            for oi in range(4):
                nc.vector.tensor_tensor(ysb0[:, oi, :], yps[oi], rbb, op=OP.mult)
                nc.sync.dma_start(out=yv[:, oi, :], in_=ysb0[:, oi, :])
            # group 1: everything resident -> ot-outer so scales/stores stream
            for oi in range(4):
                yp2 = psp.tile([128, 512], F32, tag="ps", name="yps")
                for ct in range(8):
                    nc.tensor.matmul(
                        yp2, lhsT=wo[:, ct, 512 + oi * 128:512 + (oi + 1) * 128],
                        rhs=u[:, ct, :], start=(ct == 0), stop=(ct == 7))
                nc.vector.tensor_tensor(ysb1[:, oi, :], yp2, rbb, op=OP.mult)
                nc.sync.dma_start(out=yv[:, 4 + oi, :], in_=ysb1[:, oi, :])
    nc.compile()
    return nc


def _get(name):
    if name not in _cache:
        _cache[name] = build_gsa() if name == "gsa" else build_final()
    return _cache[name]


def kernel(hidden_states, Wq, Wk, Wv, Wf, g_w, Wo, _trace=False):
    bf = ml_dtypes.bfloat16
    e4 = ml_dtypes.float8_e4m3
    hidden_states = np.asarray(hidden_states, np.float32)
    Wq, Wk, Wv, Wf = (np.asarray(x, np.float32) for x in (Wq, Wk, Wv, Wf))
    g_w, Wo = np.asarray(g_w, np.float32), np.asarray(Wo, np.float32)

    mask = np.triu(np.ones((128, 128), np.float32)).astype(bf)  # keep l <= tau
    ident = np.eye(128).astype(bf)
    in1 = []
    for core in range(8):
        b, h = core // 4, core % 4
        sl = slice(h * 256, (h + 1) * 256)
        wv128 = Wv[sl].T * WS
        wv8 = wv128.astype(e4).astype(np.float32)
        w8 = np.concatenate(
            [Wq[sl].T * WS, Wk[sl].T * WS, Wf[sl].T * WS,
             wv8, Wv[sl].T * 4.0, wv128 - wv8], axis=1)     # [1024, 1536]
        hst = np.ascontiguousarray(hidden_states[b].T)
        hs8 = hst.astype(e4)
        hr8 = ((hst - hs8.astype(np.float32)) * 32.0).astype(e4)
        in1.append({
            "hs8": hs8,
            "hr8": hr8,
            "w8": np.ascontiguousarray(w8).astype(e4),
            "mask": mask,
            "ident": ident,
        })
    nc1 = _get("gsa")
    r1 = bass_utils.run_bass_kernel_spmd(nc1, in1, core_ids=list(range(8)),
                                         trace=_trace)
    zs = [r1.results[c]["z"] for c in range(8)]        # each [256, 2048] bf16

    # wot folds g_w and the 1/512 that de-scales u = 512*silu(o)
    wot = np.ascontiguousarray((Wo * (g_w / 512.0)[None, :]).T).astype(bf)
    in2 = []
    for core in range(8):
        b, q = core // 4, core % 4
        zb = np.concatenate([zs[b * 4 + hh] for hh in range(4)], axis=0)
        in2.append({
            "zin": np.ascontiguousarray(zb[:, q * 512:(q + 1) * 512]),
            "wot": wot,
        })
    nc2 = _get("final")
    r2 = bass_utils.run_bass_kernel_spmd(nc2, in2, core_ids=list(range(8)),
                                         trace=_trace)
    out = np.empty((B, T, D), np.float32)
    for core in range(8):
        b, q = core // 4, core % 4
        out[b, q * 512:(q + 1) * 512, :] = np.asarray(
            r2.results[core]["y"], np.float32).T
    if _trace:
        kernel.last_traces = (r1, r2)
    return out


# revision 57
# speedup vs baseline: 1.0056x; 1.0056x over previous
"""Gated Slot Attention (GSA) Trainium2 kernel, v7.

Sharding: B*H = 8 lanes -> 8 cores (core = b*4 + h). Each core computes its
lane's projections + chunked two-pass GLA recurrence, emitting the raw lane
output z = 256*o transposed [DV, T]. A second kernel applies silu + RMSNorm +
output projection with rows of (b,t) split across cores.

Key design points (cost-model driven, hardware-verified):
  - q/k/f projections run in fp8e4m3 with perf_mode=DoubleRow (weights
    host-prescaled by 128 to clear the e4m3 subnormal range; the scales are
    folded into the tanh/exp activation scales and the softmax exp scale).
  - v projection is fp8 DoubleRow with residual correction: three terms
    hs8@wv8 + hr8@wv4 + hs8@wvr8 accumulate in one psum at scale 128;
    the T1+T3 pair cancels the wv quantization exactly, hr8 carries the
    hs quantization residual (x32). Measured ~1.1e-3 on v vs 2.6e-2 for
    naive fp8 - the value path is the error-critical one.
  - chunk pipeline stages P(transposes+state), Q(logits+exp), R(softmax
    denom), S(pass-2 output) run at lags 0/1/2/3, interleaved with the
    later projection batches so the PE queue never blocks on input DMAs.
  - the softmax denominator factors out of both pass-2 matmul groups
    (they are linear in qtt), so S consumes the unnormalized lam*et and
    the reciprocal-denominator broadcast multiplies the z output tile.
  - Lam_end broadcasts built batched via diag trick: diag = ident*lend_col
    (per-partition tensor_scalar), then ones^T @ diag broadcasts along free.
  - mask applied blockwise: only the two diagonal 128x128 blocks need the
    triangular multiply (in-place on SBUF after one Act copy); the zero
    block is skipped via partial-psum accumulation in the consumers.
  - chunk psums are merged into [128,512] banks ([Hk-half | Hv-half],
    [ok mt0 | ok mt1], ...) so each psum->sbuf crossing is one op.
  - GPSIMD (Pool) cannot touch PSUM and cannot run TensorScalarPtr-class
    ops (scan/stt/tensor_scalar) on this compiler: Pool only gets SBUF
    TensorTensor/copies (stb subtract, tmp2 = lam*et, p2m tri masks).

Chunked recurrence (C=256, all within one lane):
  Lam[i,m] = prod_{j<=i} g[j,m]  (= exp(-cumsum(softplus(-xf))/8))
  rlam = 1/Lam ; st_t = s_t/Lam_t = rlam_t - rlam_{t-1}
  ok   = Lam*(q @ Hk + mask(k^T q)^T St); qv = softmax_m(ok); qtt = qv*Lam
  o    = qtt @ Hv + mask(St qtt)^T v
  Hk' = Lend*(Hk + k^T St) ; Hv' = Lend*(Hv + St^T v)   (Lend pulled out)

silu is synthesized as 2*silu(x) = (tanh(x/2)+1)*x on the 128x-prescaled
psums, so q,k,v all carry 256*silu; q*k's 2^16 cancels in the exp scale,
v's 256 rides into kernel 2 where u = (tanh(z/512)+1)*z = 512*silu(o),
the RMSNorm scale uses 2^-28, and wot folds g_w/512.
"""
import sys
sys.path.insert(0, '/opt/trn_rl_repo')

import numpy as np
import ml_dtypes

import concourse.bass as bass
import concourse.bacc as bacc
import concourse.tile as tile
import concourse.mybir as mybir
import concourse.bass_utils as bass_utils

BF = mybir.dt.bfloat16
F32 = mybir.dt.float32
E4 = mybir.dt.float8e4
AF = mybir.ActivationFunctionType
OP = mybir.AluOpType
DR = mybir.MatmulPerfMode.DoubleRow

B, T, D = 2, 2048, 1024
H, DK, DV, M = 4, 256, 256, 256
C = 256            # chunk length
NCHUNK = T // C
NBATCH = NCHUNK // 2   # 2-chunk projection batches
GATE_NORM = 8.0
EPS = 1e-5
WS = 128.0         # fp8 weight prescale

_cache = {}


def build_gsa():
    """Kernel 1: per-lane projections + chunked GLA. Output z [256, 2048] bf16
    (= 2*o, feature-major)."""
    nc = bacc.Bacc("TRN2", target_bir_lowering=False, debug=False, num_devices=8)
    hs8_d = nc.dram_tensor("hs8", [D, T], E4, kind="ExternalInput").ap()
    hr8_d = nc.dram_tensor("hr8", [D, T], E4, kind="ExternalInput").ap()
    w8_d = nc.dram_tensor("w8", [D, 1536], E4, kind="ExternalInput").ap()
    mask_d = nc.dram_tensor("mask", [128, 128], BF, kind="ExternalInput").ap()
    ident_d = nc.dram_tensor("ident", [128, 128], BF, kind="ExternalInput").ap()
    z_d = nc.dram_tensor("z", [DV, T], BF, kind="ExternalOutput").ap()

    with tile.TileContext(nc) as tc:
        with (
            tc.tile_pool(name="persist", bufs=1) as pp,
            tc.tile_pool(name="hs8p", bufs=NBATCH) as hs8p,
            tc.tile_pool(name="hsbp", bufs=NBATCH) as hsbp,
            tc.tile_pool(name="gb", bufs=2) as gb,      # gate short-lived
            tc.tile_pool(name="gk", bufs=NBATCH) as gk,  # gate kept
            tc.tile_pool(name="qk", bufs=NBATCH) as qkp,
            tc.tile_pool(name="lv", bufs=8) as lv,      # per-chunk leaf tensors
            tc.tile_pool(name="l4", bufs=5) as l4,      # shorter-lived leafs
            tc.tile_pool(name="sn", bufs=6) as snp,     # state snapshots
            tc.tile_pool(name="wk", bufs=4) as wk,      # short-lived
            tc.tile_pool(name="p512", bufs=2, space="PSUM") as p512,
            tc.tile_pool(name="pC", bufs=3, space="PSUM") as pC,
            tc.tile_pool(name="pT", bufs=2, space="PSUM") as pT,
            tc.tile_pool(name="pR", bufs=1, space="PSUM") as pR,
        ):
            w8 = pp.tile([128, 8, 1536], E4, tag="w8")
            msk = pp.tile([128, 128], BF, tag="msk")
            ident = pp.tile([128, 128], BF, tag="ident")
            ones_col = pp.tile([128, 1], BF, tag="onescol")
            ones_row = pp.tile([1, 128], BF, tag="onesrow")
            ones_mat = pp.tile([128, 128], BF, tag="onesmat")
            ones_2h = pp.tile([128, 256], BF, tag="ones2h")
            e1all = pp.tile([128, 8, 512], F32, tag="e1all")

            w8v = w8_d.rearrange("(a p) o -> p a o", p=128)
            hsv8 = hs8_d.rearrange("(a p) t -> p a t", p=128)
            hsv = hr8_d.rearrange("(a p) t -> p a t", p=128)
            # f weights first: the gate phase runs before everything else.
            nc.sync.dma_start(out=w8[:, :, 512:768], in_=w8v[:, :, 512:768])
            hs8_t, hsb_t = {}, {}
            for bt in range(NBATCH):
                hs8_t[bt] = hs8p.tile([128, 8, 512], E4, tag="hs8", name="hs8")
            for bt in range(NBATCH):
                hsb_t[bt] = hsbp.tile([128, 8, 512], E4, tag="hr8", name="hr8")
            nc.sync.dma_start(out=hs8_t[0], in_=hsv8[:, :, 0:512])
            nc.sync.dma_start(out=w8[:, :, 0:512], in_=w8v[:, :, 0:512])
            for bt in range(1, NBATCH):
                nc.sync.dma_start(out=hs8_t[bt],
                                  in_=hsv8[:, :, bt * 512:(bt + 1) * 512])
            nc.sync.dma_start(out=msk, in_=mask_d)
            nc.sync.dma_start(out=ident, in_=ident_d)
            nc.sync.dma_start(out=w8[:, :, 768:1536], in_=w8v[:, :, 768:1536])
            for bt in range(NBATCH):
                nc.sync.dma_start(out=hsb_t[bt],
                                  in_=hsv[:, :, bt * 512:(bt + 1) * 512])
            nc.vector.memset(ones_col, 1.0)
            nc.vector.memset(ones_row, 1.0)
            nc.gpsimd.memset(ones_mat, 1.0)
            nc.gpsimd.memset(ones_2h, 1.0)
            # warm the PE pstate during the initial DMA wait: tiny matmuls
            # keep the tensor engine busy so the first real projections run
            # at full clock instead of the mid pstate
            wps = pR.tile([128, 512], F32, tag="pR", name="wps")
            for _ in range(40):
                nc.tensor.matmul(wps[0:1, 0:1], lhsT=ones_col, rhs=ones_col,
                                 start=True, stop=True, skip_group_check=True)

            zv = z_d.rearrange("(a p) t -> p a t", p=128)

            lendf = pp.tile([128, 2, NCHUNK], F32, tag="lendf")
            lamb, stb, qtb, ktb = {}, {}, {}, {}
            v_un, st_un, k_un, hxb, et, tmp2, qtt = ({} for _ in range(7))

            # ---- phase F: f projections (fp8 DoubleRow) + exp for all
            # batches; then one Ln for softplus; then per-batch gate math.
            for bt in range(NBATCH):
                hs8 = hs8_t[bt]
                for mt in range(2):
                    ps = p512.tile([128, 512], F32, tag="p512")
                    for d2 in range(4):
                        nc.tensor.matmul(
                            ps,
                            lhsT=w8[:, 2 * d2:2 * d2 + 2,
                                    512 + mt * 128:512 + (mt + 1) * 128],
                            rhs=hs8[:, 2 * d2:2 * d2 + 2, :],
                            perf_mode=DR, start=(d2 == 0), stop=(d2 == 3))
                    # e1 = exp(-xf); psum carries 128*xf
                    nc.scalar.activation(e1all[:, bt * 2 + mt, :], ps, AF.Exp,
                                         scale=-1.0 / WS)
            # ln(e1 + 1) = softplus(-xf) = nsp, all batches in one instruction
            nc.scalar.activation(e1all, e1all, AF.Ln, bias=1.0)

            def stage_F3(bt):
                """Post-ln gate math for one batch: cumsum, rlam, lam, st."""
                e1 = e1all[:, bt * 2:bt * 2 + 2, :]
                rl = gb.tile([128, 2, 512], F32, tag="rl", name="rl")
                Sb = gb.tile([128, 2, 512], F32, tag="Sb", name="Sb")
                lamb[bt] = gk.tile([128, 2, 512], BF, tag="lamb", name="lamb")
                stb[bt] = gk.tile([128, 2, 512], BF, tag="stb", name="stb")
                seng = nc.vector
                for mt in range(2):
                    # per-chunk cumsum of nsp
                    seng.tensor_tensor_scan(
                        Sb[:, mt, 0:256], e1[:, mt, 0:256], e1[:, mt, 0:256],
                        0.0, OP.add, OP.bypass)
                    seng.tensor_tensor_scan(
                        Sb[:, mt, 256:512], e1[:, mt, 256:512],
                        e1[:, mt, 256:512], 0.0, OP.add, OP.bypass)
                # rlam = exp(cumsum/8), both mt in one activation
                nc.scalar.activation(
                    rl.rearrange("p a b -> p (a b)"),
                    Sb.rearrange("p a b -> p (a b)"), AF.Exp,
                    scale=1.0 / GATE_NORM)
                with nc.allow_low_precision(reason="lam in bf16"):
                    nc.vector.reciprocal(
                        lamb[bt].rearrange("p a b -> p (a b)"),
                        rl.rearrange("p a b -> p (a b)"))
                for mt in range(2):
                    for h2 in range(2):
                        nc.vector.tensor_copy(
                            lendf[:, mt, 2 * bt + h2:2 * bt + h2 + 1],
                            lamb[bt][:, mt, h2 * 256 + 255:h2 * 256 + 256])
                    # st_t = rlam_t - rlam_{t-1}; chunk-boundary cols use rlam=1
                    nc.gpsimd.tensor_tensor(
                        stb[bt][:, mt, 1:512], rl[:, mt, 1:512],
                        rl[:, mt, 0:511], op=OP.subtract)
                    for h2 in range(2):
                        nc.vector.tensor_scalar_sub(
                            stb[bt][:, mt, h2 * 256:h2 * 256 + 1],
                            rl[:, mt, h2 * 256:h2 * 256 + 1], 1.0)


            def chunk_views(c):
                bt, h2 = c // 2, c % 2
                off = h2 * 256
                stc = stb[bt][:, :, off:off + 256]
                lamc = lamb[bt][:, :, off:off + 256]
                qtc = qtb[bt][:, :, off:off + 256]
                ktc = ktb[bt][:, :, off:off + 256]
                return stc, lamc, qtc, ktc

            # ---- phase G: q/k (fp8 DoubleRow) + v (bf16) projections + silu.
            def stage_Gqk(bt):
                hs8 = hs8_t[bt]
                qtb[bt] = qkp.tile([128, 2, 512], BF, tag="qtb", name="qtb")
                ktb[bt] = qkp.tile([128, 2, 512], BF, tag="ktb", name="ktb")
                for base, dst in ((0, qtb[bt]), (256, ktb[bt])):
                    for ot in range(2):
                        ps = p512.tile([128, 512], F32, tag="p512")
                        for d2 in range(4):
                            nc.tensor.matmul(
                                ps,
                                lhsT=w8[:, 2 * d2:2 * d2 + 2,
                                        base + ot * 128:base + (ot + 1) * 128],
                                rhs=hs8[:, 2 * d2:2 * d2 + 2, :],
                                perf_mode=DR, start=(d2 == 0), stop=(d2 == 3))
                        th = wk.tile([128, 512], BF, tag="th")
                        # psum = 128*x -> th = tanh(x/2); dst = (th+1)*psum
                        # = 256*silu(x)
                        nc.scalar.activation(th, ps, AF.Tanh, scale=0.5 / WS)
                        nc.vector.scalar_tensor_tensor(
                            out=dst[:, ot, :], in0=th, scalar=1.0, in1=ps,
                            op0=OP.add, op1=OP.mult)
            def stage_Gv(bt):
                """v = hs8@wv8 + hr8@wv8b + hs8@wvr8, all fp8 DoubleRow at
                psum scale 128 (T1+T3 cancel the wv quantization exactly)."""
                hs8, hr8 = hs8_t[bt], hsb_t[bt]
                for h2 in range(2):
                    c = 2 * bt + h2
                    v_un[c] = lv.tile([128, 2, 256], BF, tag="vun", name="vun",
                                      bufs=8)
                    psw = p512.tile([128, 512], F32, tag="p512")
                    for tt in range(2):
                        ps = psw[:, tt * 256:(tt + 1) * 256]
                        tsl = slice(h2 * 256 + tt * 128, h2 * 256 + (tt + 1) * 128)
                        for d2 in range(4):
                            d2s = slice(2 * d2, 2 * d2 + 2)
                            nc.tensor.matmul(
                                ps, lhsT=hs8[:, d2s, tsl],
                                rhs=w8[:, d2s, 768:1024], perf_mode=DR,
                                start=(d2 == 0), stop=False,
                                skip_group_check=True)
                        for d2 in range(4):
                            d2s = slice(2 * d2, 2 * d2 + 2)
                            nc.tensor.matmul(
                                ps, lhsT=hr8[:, d2s, tsl],
                                rhs=w8[:, d2s, 1024:1280], perf_mode=DR,
                                start=False, stop=False, skip_group_check=True)
                        for d2 in range(4):
                            d2s = slice(2 * d2, 2 * d2 + 2)
                            nc.tensor.matmul(
                                ps, lhsT=hs8[:, d2s, tsl],
                                rhs=w8[:, d2s, 1280:1536], perf_mode=DR,
                                start=False, stop=(d2 == 3),
                                skip_group_check=True)
                    th = wk.tile([128, 512], BF, tag="th")
                    # psum = 128*v -> v_un = 256*silu(v)
                    nc.scalar.activation(th, psw, AF.Tanh, scale=0.5 / WS)
                    nc.vector.scalar_tensor_tensor(
                        out=v_un[c].rearrange("p a b -> p (a b)"), in0=th,
                        scalar=1.0, in1=psw, op0=OP.add, op1=OP.mult)

            # Batched Lend broadcast: diag trick
            # dg[p,j] = ident[p,j]*Lend[p]; ones^T @ dg -> [128, Lend[j]].
            # One round covers the 2 chunks of one batch.
            lbc_all = pp.tile([128, NCHUNK - 1, 256], BF, tag="lbcall")

            def stage_prep(r):
                pbc = pR.tile([128, 512], F32, tag="pR")
                dgr = wk.tile([128, 4, 128], BF, tag="dgr")
                nch = min(2, NCHUNK - 1 - 2 * r)
                for i in range(nch):
                    c = 2 * r + i
                    for mt in range(2):
                        nc.vector.tensor_scalar_mul(
                            dgr[:, 2 * i + mt, :], ident,
                            lendf[:, mt, c:c + 1])
                        nc.tensor.matmul(
                            pbc[:, i * 256 + mt * 128:i * 256 + (mt + 1) * 128],
                            lhsT=ones_mat, rhs=dgr[:, 2 * i + mt, :],
                            start=True, stop=True, skip_group_check=True)
                nc.scalar.activation(
                    lbc_all[:, 2 * r:2 * r + nch, :].rearrange(
                        "p a b -> p (a b)"),
                    pbc[:, 0:nch * 256], AF.Copy)

            def stage_P(c):
                """Transposes, merged state updates."""
                stc, lamc, qtc, ktc = chunk_views(c)
                # transposes: [tau, m | dk]: skun[:,lt,0:256]=st_un, 256:512=k_un
                skun = lv.tile([128, 2, 512], BF, tag="skun", name="skun")
                st_un[c] = skun[:, :, 0:256]
                k_un[c] = skun[:, :, 256:512]
                pst = pT.tile([128, 1024], BF, tag="pT")
                for lt in range(2):
                    for mt in range(2):
                        nc.tensor.transpose(
                            pst[:, lt * 512 + mt * 128:lt * 512 + (mt + 1) * 128],
                            stc[:, mt, lt * 128:(lt + 1) * 128], ident)
                    for k2 in range(2):
                        nc.tensor.transpose(
                            pst[:, lt * 512 + 256 + k2 * 128:
                                lt * 512 + 256 + (k2 + 1) * 128],
                            ktc[:, k2, lt * 128:(lt + 1) * 128], ident)
                nc.scalar.activation(
                    skun.rearrange("p a b -> p (a b)"), pst, AF.Copy)

                if c >= NCHUNK - 1:
                    return
                # merged state psums: psx = [Hk(dt2=x) | Hv(mt=x)] [128,512]
                first = c == 0
                hxb[c] = [snp.tile([128, 512], BF, tag=f"hxb{x}", name="hxb")
                          for x in range(2)]
                for x in range(2):
                    ps = pC.tile([128, 512], F32, tag="pC")
                    for lt in range(2):
                        nc.tensor.matmul(
                            ps[:, 0:256],
                            lhsT=k_un[c][:, lt, x * 128:(x + 1) * 128],
                            rhs=st_un[c][:, lt, :], start=(lt == 0),
                            stop=False, skip_group_check=True)
                    for lt in range(2):
                        nc.tensor.matmul(
                            ps[:, 256:512],
                            lhsT=st_un[c][:, lt, x * 128:(x + 1) * 128],
                            rhs=v_un[c][:, lt, :], start=(lt == 0),
                            stop=(lt == 1 and first), skip_group_check=True)
                    if not first:
                        nc.tensor.matmul(ps, lhsT=ident, rhs=hxb[c - 1][x],
                                         start=False, stop=True,
                                         skip_group_check=True)
                    nc.vector.tensor_tensor(hxb[c][x][:, 0:256], ps[:, 0:256],
                                            lbc_all[:, c, :], op=OP.mult)
                    nc.vector.tensor_scalar_mul(hxb[c][x][:, 256:512],
                                                ps[:, 256:512],
                                                lendf[:, x, c:c + 1])

            def hk_v(c, k2):
                return hxb[c][k2][:, 0:256]

            def hv_v(c, mt):
                return hxb[c][mt][:, 256:512]

            def stage_Q(c):
                """Gram + masked intra + state ok + exp for chunk c (lag 1)."""
                stc, lamc, qtc, ktc = chunk_views(c)
                # ptm[lambda, tau] = mask * (k^T q); blockwise, merged psum.
                ptm = l4.tile([128, 2, 256], BF, tag="ptm", name="ptm")
                psg = pC.tile([128, 512], F32, tag="pC")
                for lt in range(2):
                    for k2 in range(2):
                        nc.tensor.matmul(
                            psg[:, lt * 256:(lt + 1) * 256],
                            lhsT=ktc[:, k2, lt * 128:(lt + 1) * 128],
                            rhs=qtc[:, k2, :], start=(k2 == 0), stop=(k2 == 1),
                            skip_group_check=True)
                nc.scalar.activation(
                    ptm.rearrange("p a b -> p (a b)"), psg, AF.Copy)
                nc.vector.tensor_tensor(ptm[:, 0, 0:128], ptm[:, 0, 0:128],
                                        msk, op=OP.mult)
                nc.vector.tensor_tensor(ptm[:, 1, 128:256], ptm[:, 1, 128:256],
                                        msk, op=OP.mult)
                tmp = wk.tile([128, 2, 256], F32, tag="tmp")
                et[c] = l4.tile([128, 2, 256], BF, tag="et", name="et")
                first = c == 0
                pso = pC.tile([128, 512], F32, tag="pC")
                for mt in range(2):
                    ob = pso[:, mt * 256:(mt + 1) * 256]
                    if not first:
                        for k2 in range(2):
                            nc.tensor.matmul(
                                ob, lhsT=hk_v(c - 1, k2)[:, mt * 128:(mt + 1) * 128],
                                rhs=qtc[:, k2, :], start=(k2 == 0), stop=False,
                                skip_group_check=True)
                    nc.tensor.matmul(
                        ob, lhsT=st_un[c][:, 0, mt * 128:(mt + 1) * 128],
                        rhs=ptm[:, 0, :], start=first, stop=False,
                        skip_group_check=True)
                    nc.tensor.matmul(
                        ob[:, 128:256],
                        lhsT=st_un[c][:, 1, mt * 128:(mt + 1) * 128],
                        rhs=ptm[:, 1, 128:256], start=False, stop=(mt == 1),
                        skip_group_check=True)
                nc.vector.tensor_tensor(
                    tmp, pso.rearrange("p (a b) -> p a b", a=2),
                    lamc, op=OP.mult)
                # q,k each carry 256x -> exp scale 2^-16
                nc.scalar.activation(
                    et[c].rearrange("p a b -> p (a b)"),
                    tmp.rearrange("p a b -> p (a b)"), AF.Exp,
                    scale=1.0 / 65536.0)
                # unnormalized qtt = lam * et (normalized in stage_R)
                tmp2[c] = l4.tile([128, 2, 256], BF, tag="tmp2", name="tmp2")
                nc.gpsimd.tensor_tensor(tmp2[c], lamc, et[c], op=OP.mult)

            def stage_R(c):
                """Softmax normalization for chunk c (lag 2)."""
                prt = pR.tile([128, 512], F32, tag="pR")
                cs = prt[0:1, 256:512]
                for mt in range(2):
                    nc.tensor.matmul(cs, lhsT=ones_col, rhs=et[c][:, mt, :],
                                     start=(mt == 0), stop=(mt == 1),
                                     skip_group_check=True)
                rrow = wk.tile([1, C], BF, tag="rrow")
                with nc.allow_low_precision(reason="softmax denom bcast in bf16"):
                    nc.vector.reciprocal(rrow, cs)
                bcr = prt[:, 0:256]
                nc.tensor.matmul(bcr, lhsT=ones_row, rhs=rrow, start=True,
                                 stop=True, skip_group_check=True)
                rbb = l4.tile([128, C], BF, tag="rbb", name="rbb")
                nc.scalar.activation(rbb, bcr, AF.Copy)
                qtt[c] = rbb  # denominator factor, applied to z in stage_S

            def stage_S(c):
                """Pass-2 output for chunk c (lag 3)."""
                stc, lamc, qtc, ktc = chunk_views(c)
                # p2m[lambda, tau] = mask * (St qtt); blockwise, merged psum.
                p2m = wk.tile([128, 2, C], BF, tag="p2m")
                ps2 = pC.tile([128, 512], F32, tag="pC")
                for mt in range(2):
                    nc.tensor.matmul(
                        ps2[:, 0:256], lhsT=stc[:, mt, 0:128],
                        rhs=tmp2[c][:, mt, :], start=(mt == 0), stop=False,
                        skip_group_check=True)
                for mt in range(2):
                    nc.tensor.matmul(
                        ps2[:, 384:512], lhsT=stc[:, mt, 128:256],
                        rhs=tmp2[c][:, mt, 128:256], start=(mt == 0),
                        stop=(mt == 1), skip_group_check=True)
                nc.scalar.activation(
                    p2m.rearrange("p a b -> p (a b)"), ps2, AF.Copy)
                nc.gpsimd.tensor_tensor(p2m[:, 0, 0:128], p2m[:, 0, 0:128],
                                        msk, op=OP.mult)
                nc.gpsimd.tensor_tensor(p2m[:, 1, 128:256], p2m[:, 1, 128:256],
                                        msk, op=OP.mult)

                first = c == 0
                zps = p512.tile([128, 512], F32, tag="p512")
                for vt in range(2):
                    zp = zps[:, vt * 256:(vt + 1) * 256]
                    nc.tensor.matmul(
                        zp, lhsT=v_un[c][:, 0, 128 * vt:128 * (vt + 1)],
                        rhs=p2m[:, 0, :], start=True, stop=False,
                        skip_group_check=True)
                    nc.tensor.matmul(
                        zp[:, 128:256],
                        lhsT=v_un[c][:, 1, 128 * vt:128 * (vt + 1)],
                        rhs=p2m[:, 1, 128:256], start=False,
                        stop=(first and vt == 1), skip_group_check=True)
                    if not first:
                        for mt in range(2):
                            nc.tensor.matmul(
                                zp, lhsT=hv_v(c - 1, mt)[:, vt * 128:(vt + 1) * 128],
                                rhs=tmp2[c][:, mt, :], start=False,
                                stop=(mt == 1), skip_group_check=True)
                # un-normalized z; the softmax denominator (rbb) factors out
                # of both matmul groups and is applied here
                zsb = wk.tile([128, 2, C], BF, tag="zsb")
                for vt in range(2):
                    nc.vector.tensor_tensor(
                        zsb[:, vt, :], zps[:, vt * 256:(vt + 1) * 256],
                        qtt[c], op=OP.mult)
                nc.sync.dma_start(out=zv[:, :, c * C:(c + 1) * C], in_=zsb)

            stage_F3(0)
            stage_prep(0)
            stage_Gqk(0)
            stage_Gv(0)
            stage_F3(1)
            stage_prep(1)
            stage_Gqk(1)
            for it in range(NCHUNK + 3):
                if it == 0:
                    stage_Gv(1)
                if it == 2:
                    stage_F3(2)
                    stage_prep(2)
                    stage_Gqk(2)
                    stage_Gv(2)
                if it == 4:
                    stage_F3(3)
                    stage_prep(3)
                    stage_Gqk(3)
                    stage_Gv(3)
                if it < NCHUNK:
                    stage_P(it)
                if 1 <= it < NCHUNK + 1:
                    stage_Q(it - 1)
                if 2 <= it < NCHUNK + 2:
                    stage_R(it - 2)
                if 3 <= it < NCHUNK + 3:
                    stage_S(it - 3)
    nc.compile()
    return nc


def build_final():
    """Kernel 2: z = 256*o; u = (tanh(z/512)+1)*z = 512*silu(o);
    y = (u * rsqrt-bcast) @ wot, wot host-folded (g_w, 1/512).
    Out yT [1024, 512] bf16."""
    nc = bacc.Bacc("TRN2", target_bir_lowering=False, debug=False, num_devices=8)
    z_d = nc.dram_tensor("zin", [D, 512], BF, kind="ExternalInput").ap()
    wo_d = nc.dram_tensor("wot", [D, D], BF, kind="ExternalInput").ap()
    y_d = nc.dram_tensor("y", [D, 512], BF, kind="ExternalOutput").ap()

    with tile.TileContext(nc) as tc:
        with (
            tc.tile_pool(name="sb", bufs=1) as sb,
            tc.tile_pool(name="yp", bufs=3) as yp,
            tc.tile_pool(name="ps", bufs=6, space="PSUM") as psp,
            tc.tile_pool(name="pss", bufs=1, space="PSUM") as pssp,
            tc.tile_pool(name="psb", bufs=1, space="PSUM") as psbp,
        ):
            z = sb.tile([128, 8, 512], BF, tag="z")
            wo = sb.tile([128, 8, 1024], BF, tag="wo")
            u = sb.tile([128, 8, 512], BF, tag="u")
            squ = sb.tile([128, 8, 512], BF, tag="squ")
            ysb0 = sb.tile([128, 4, 512], BF, tag="ysb0")
            ysb1 = sb.tile([128, 4, 512], BF, tag="ysb1")
            ones_col = sb.tile([128, 1], BF, tag="onescol")
            ones_row = sb.tile([1, 128], BF, tag="onesrow")
            nc.vector.memset(ones_col, 1.0)
            nc.vector.memset(ones_row, 1.0)
            zvw = z_d.rearrange("(a p) t -> p a t", p=128)
            wov = wo_d.rearrange("(a p) o -> p a o", p=128)
            for ct in range(8):
                nc.sync.dma_start(out=z[:, ct, :], in_=zvw[:, ct, :])
                nc.sync.dma_start(out=wo[:, ct, :], in_=wov[:, ct, :])

            # silu: z = 2*o ; u = (tanh(o/2)+1)*z = 4*silu(o)
            sqs = pssp.tile([1, 512], F32, tag="pss")
            yps = [psp.tile([128, 512], F32, tag="ps", name="yps")
                   for _ in range(4)]
            for ct in range(8):
                th = yp.tile([128, 512], BF, tag="th")
                nc.scalar.activation(th, z[:, ct, :], AF.Tanh,
                                     scale=1.0 / 512.0)
                nc.vector.scalar_tensor_tensor(
                    out=u[:, ct, :], in0=th, scalar=1.0, in1=z[:, ct, :],
                    op0=OP.add, op1=OP.mult)
                nc.vector.tensor_tensor(squ[:, ct, :], u[:, ct, :], u[:, ct, :],
                                        op=OP.mult)
                nc.tensor.matmul(sqs, lhsT=ones_col, rhs=squ[:, ct, :],
                                 start=(ct == 0), stop=(ct == 7))
                for oi in range(4):
                    nc.tensor.matmul(
                        yps[oi], lhsT=wo[:, ct, oi * 128:(oi + 1) * 128],
                        rhs=u[:, ct, :], start=(ct == 0), stop=(ct == 7))
            # u = 512*silu -> mean(silu^2) = sqs/(512^2*1024) = sqs*2^-28
            sq = sb.tile([1, 512], F32, tag="sq")
            eps_t = sb.tile([1, 1], F32, tag="epst")
            nc.vector.memset(eps_t, EPS)
            nc.scalar.activation(sq, sqs, AF.Sqrt, scale=2.0 ** -28,
                                 bias=eps_t)
            rr = sb.tile([1, 512], BF, tag="rr")
            with nc.allow_low_precision(reason="rms bcast in bf16"):
                nc.vector.reciprocal(rr, sq)
            bcr = psbp.tile([128, 512], F32, tag="psb")
            nc.tensor.matmul(bcr, lhsT=ones_row, rhs=rr, start=True, stop=True)
            rbb = sb.tile([128, 512], F32, tag="rbb")
            nc.vector.tensor_copy(rbb, bcr)

            yv = y_d.rearrange("(a p) t -> p a t", p=128)
            for oi in range(4):
                nc.vector.tensor_tensor(ysb0[:, oi, :], yps[oi], rbb, op=OP.mult)
                nc.sync.dma_start(out=yv[:, oi, :], in_=ysb0[:, oi, :])
            # group 1: everything resident -> ot-outer so scales/stores stream
            for oi in range(4):
                yp2 = psp.tile([128, 512], F32, tag="ps", name="yps")
                for ct in range(8):
                    nc.tensor.matmul(
                        yp2, lhsT=wo[:, ct, 512 + oi * 128:512 + (oi + 1) * 128],
                        rhs=u[:, ct, :], start=(ct == 0), stop=(ct == 7))
                nc.vector.tensor_tensor(ysb1[:, oi, :], yp2, rbb, op=OP.mult)
                nc.sync.dma_start(out=yv[:, 4 + oi, :], in_=ysb1[:, oi, :])
    nc.compile()
    return nc


def _get(name):
    if name not in _cache:
        _cache[name] = build_gsa() if name == "gsa" else build_final()
    return _cache[name]


def kernel(hidden_states, Wq, Wk, Wv, Wf, g_w, Wo, _trace=False):
    bf = ml_dtypes.bfloat16
    e4 = ml_dtypes.float8_e4m3
    hidden_states = np.asarray(hidden_states, np.float32)
    Wq, Wk, Wv, Wf = (np.asarray(x, np.float32) for x in (Wq, Wk, Wv, Wf))
    g_w, Wo = np.asarray(g_w, np.float32), np.asarray(Wo, np.float32)

    mask = np.triu(np.ones((128, 128), np.float32)).astype(bf)  # keep l <= tau
    ident = np.eye(128).astype(bf)
    in1 = []
    for core in range(8):
        b, h = core // 4, core % 4
        sl = slice(h * 256, (h + 1) * 256)
        wv128 = Wv[sl].T * WS
        wv8 = wv128.astype(e4).astype(np.float32)
        w8 = np.concatenate(
            [Wq[sl].T * WS, Wk[sl].T * WS, Wf[sl].T * WS,
             wv8, Wv[sl].T * 4.0, wv128 - wv8], axis=1)     # [1024, 1536]
        hst = np.ascontiguousarray(hidden_states[b].T)
        hs8 = hst.astype(e4)
        hr8 = ((hst - hs8.astype(np.float32)) * 32.0).astype(e4)
        in1.append({
            "hs8": hs8,
            "hr8": hr8,
            "w8": np.ascontiguousarray(w8).astype(e4),
            "mask": mask,
            "ident": ident,
        })
    nc1 = _get("gsa")
    r1 = bass_utils.run_bass_kernel_spmd(nc1, in1, core_ids=list(range(8)),
                                         trace=_trace)
    zs = [r1.results[c]["z"] for c in range(8)]        # each [256, 2048] bf16

    # wot folds g_w and the 1/512 that de-scales u = 512*silu(o)
    wot = np.ascontiguousarray((Wo * (g_w / 512.0)[None, :]).T).astype(bf)
    in2 = []
    for core in range(8):
        b, q = core // 4, core % 4
        zb = np.concatenate([zs[b * 4 + hh] for hh in range(4)], axis=0)
        in2.append({
            "zin": np.ascontiguousarray(zb[:, q * 512:(q + 1) * 512]),
            "wot": wot,
        })
    nc2 = _get("final")
    r2 = bass_utils.run_bass_kernel_spmd(nc2, in2, core_ids=list(range(8)),
                                         trace=_trace)
    out = np.empty((B, T, D), np.float32)
    for core in range(8):
        b, q = core // 4, core % 4
        out[b, q * 512:(q + 1) * 512, :] = np.asarray(
            r2.results[core]["y"], np.float32).T
    if _trace:
        kernel.last_traces = (r1, r2)
    return out
